# revision 11
# baseline (speedup 1.0000x reference)
"""ConvGRU Trainium2 kernel (v4).

video [B=2, T=16, C=128, H=64, W=64] f32; 1x1-conv GRU over T.
Sharding: data-parallel over (B x H/16) -> 8 cores, each core owns
P = 16*64 = 1024 pixels for all T; weights replicated.

Per core, per timestep (pixels on the free dim, channels on partitions):
    zr_pre = [Wzx@x + Wzh@h | Wrx@x + Wrh@h]      (PE, fp16 in / fp32 psum)
    z = sigmoid(zr_pre[:P] + bz); r = sigmoid(zr_pre[P:] + br)   (ACT)
    rh = r * h                                     (DVE)
    c = tanh(Whx@x + Whh@rh + bh)                  (PE + ACT)
    h = h + z * (c - h)                            (DVE, fp16 state)

The recurrence is latency-bound: each pixel group's step is a serial
cross-engine chain (h -> Wrh matmul -> sigmoid -> r*h -> Whh matmul ->
tanh -> blend -> h').  G=2 pixel groups form two independent chains
that interleave on the engines; all per-step ops stay PER-GROUP (a
merged-op variant that coupled the chains measured 33% slower).

Changes vs the 93.1us baseline (v1):
  - t=0 shortcut: h0 == 0, so closers, r-sigmoid and rh are skipped
    and h1 = sigmoid(pre_z + bz) * tanh(pre_c + bh)
  - output DMAs ride the otherwise-idle GpSimd queue so x prefetches
    never queue behind them on the sync HW queue
  - x prefetch issued two steps ahead (DMA landing latency is about
    one whole step period)
  - weight DMA split across the sync + gpsimd queues (x-side first)
    so x0 lands earlier; PE warmup matmuls run against a memset tile
    (no weight dependency) flipping the HAM clock gate during the DMAs
  - work pool bufs=3 so WAR buffer-recycle waits are stale and cheap

Numerics: fp16 matmul inputs/gates/state, fp32 PSUM accum + fp32 bias.
"""

import os
import sys

import numpy as np

B, T, C, H, W = 2, 16, 128, 64, 64
NCORES = 8
HQ = H // 4          # 16 rows of H per core (4 H-slices x 2 batches = 8 cores)
P = HQ * W           # 1024 pixels per core
G = 2                # pixel groups per step (independent recurrence chains)
PG = P // G          # 512 pixels per group

_PROG = None


def _ensure_paths():
    for p in ("/opt/trn_rl_repo",):
        if p not in sys.path and os.path.isdir(p):
            sys.path.append(p)


def _build():
    _ensure_paths()
    import concourse.bacc as bacc
    import concourse.tile as tile
    from concourse import mybir

    f32 = mybir.dt.float32
    f16 = mybir.dt.float16
    AF = mybir.ActivationFunctionType

    nc = bacc.Bacc(
        "TRN2", target_bir_lowering=False, debug=False, num_devices=NCORES
    )
    x_dram = nc.dram_tensor("x_seq", [T, C, P], f16, kind="ExternalInput")
    w_dram = nc.dram_tensor("wmats", [C, 6 * C], f16, kind="ExternalInput")
    b_dram = nc.dram_tensor("biases", [C, 4], f32, kind="ExternalInput")
    o_dram = nc.dram_tensor("out_seq", [T, C, P], f16, kind="ExternalOutput")

    x_ap = x_dram.ap()
    w_ap = w_dram.ap()
    b_ap = b_dram.ap()
    o_ap = o_dram.ap()

    # weight order in wmats: x-side first so its DMA can land first
    WZX, WRX, WHX, WZH, WRH, WHH = range(6)

    with tile.TileContext(nc) as tc:
        with (
            tc.tile_pool(name="consts", bufs=1) as consts,
            tc.tile_pool(name="xin", bufs=4) as xpool,
            tc.tile_pool(name="state", bufs=4) as spool,
            tc.tile_pool(name="work", bufs=3) as wk,
            tc.tile_pool(name="ps", bufs=1, space="PSUM") as ps,
        ):
            wt = consts.tile([C, 6 * C], f16)
            bt = consts.tile([C, 4], f32)
            nc.sync.dma_start(bt[:], b_ap[:])
            nc.gpsimd.dma_start(wt[:, 3 * C :], w_ap[:, 3 * C :])

            def wslice(i):
                return wt[:, i * C : (i + 1) * C]

            def load_x(t):
                xt = xpool.tile([C, P], f16, tag="x")
                nc.sync.dma_start(xt[:], x_ap[t])
                return xt

            x_tiles = {0: load_x(0)}
            nc.sync.dma_start(wt[:, : 3 * C], w_ap[:, : 3 * C])
            x_tiles[1] = load_x(1)
            x_tiles[2] = load_x(2)

            # -- warmup: ramp the PE clock gate with matmuls that only
            #    depend on a memset tile, while the input DMAs fly --
            warm16 = wk.tile([C, PG], f16, tag="warm")
            nc.vector.memset(warm16[:], 0.0)
            cwarm = [None, None]
            for g in range(G):
                cwarm[g] = ps.tile(
                    [C, PG], f32, tag=f"c_{g}", bufs=2, name=f"cwarm_{g}"
                )
            for i in range(6):
                nc.tensor.matmul(
                    cwarm[i % 2][:], warm16[:, :C], warm16[:],
                    start=True, stop=True,
                )
            # preload the ACT sigmoid/tanh table early
            wtmp = wk.tile([C, PG], f16, tag="scratch")
            nc.scalar.activation(
                wtmp[:], cwarm[0][:], AF.Sigmoid, bias=bt[:, 0:1]
            )

            # ---- t = 0: h0 == 0, so no closers / r-gate / rh ----
            x0 = x_tiles[0]
            zr0 = [None, None]
            for g in range(G):
                zrt = ps.tile([C, 2 * PG], f32, tag=f"zr_{g}", name=f"zr0_{g}")
                nc.tensor.matmul(
                    zrt[:, :PG], wslice(WZX), x0[:, g * PG : (g + 1) * PG],
                    start=True, stop=True,
                )
                zr0[g] = zrt
            c0 = [None, None]
            for g in range(G):
                cp = ps.tile([C, PG], f32, tag=f"c_{g}", bufs=2)
                nc.tensor.matmul(
                    cp[:], wslice(WHX), x0[:, g * PG : (g + 1) * PG],
                    start=True, stop=True,
                )
                c0[g] = cp
            h16 = [None, None]
            for g in range(G):
                zt = wk.tile([C, PG], f16, tag=f"zb_{g}")
                nc.scalar.activation(
                    zt[:], zr0[g][:, :PG], AF.Sigmoid, bias=bt[:, 0:1]
                )
                ct = wk.tile([C, PG], f16, tag=f"c16_{g}")
                nc.scalar.activation(ct[:], c0[g][:], AF.Tanh, bias=bt[:, 2:3])
                ht = spool.tile([C, PG], f16, tag=f"h16_{g}")
                nc.vector.tensor_mul(ht[:], zt[:], ct[:])
                h16[g] = ht
                nc.sync.dma_start(
                    o_ap[0, :, g * PG : (g + 1) * PG], ht[:]
                )

            def open_zr(xt, gorder):
                """Open z|r accumulations with the x-side contributions."""
                zr_t = [None] * G
                for g in gorder:
                    xs = xt[:, g * PG : (g + 1) * PG]
                    zrt = ps.tile([C, 2 * PG], f32, tag=f"zr_{g}", bufs=1,
                                  name=f"zr_t{g}")
                    nc.tensor.matmul(
                        zrt[:, PG:], wslice(WRX), xs, start=True, stop=False
                    )
                    nc.tensor.matmul(
                        zrt[:, :PG], wslice(WZX), xs, start=True, stop=False
                    )
                    zr_t[g] = zrt
                return zr_t

            def open_c(xt, gorder):
                cp_t = [None] * G
                for g in gorder:
                    xs = xt[:, g * PG : (g + 1) * PG]
                    cp = ps.tile([C, PG], f32, tag=f"c_{g}", bufs=2,
                                 name=f"c_t{g}")
                    nc.tensor.matmul(
                        cp[:], wslice(WHX), xs, start=True, stop=False
                    )
                    cp_t[g] = cp
                return cp_t

            first = list(range(G))
            x1 = x_tiles[1]
            zr_t = open_zr(x1, first)
            cp_t = open_c(x1, first)

            for t in range(1, T):
                go = first
                x_next = x_tiles.get(t + 1)
                if t + 2 < T:
                    x_tiles[t + 2] = load_x(t + 2)

                # -- PE: close the r then z accumulations (chain head) --
                for g in go:
                    nc.tensor.matmul(
                        zr_t[g][:, PG:], wslice(WRH), h16[g][:],
                        start=False, stop=True,
                    )
                for g in go:
                    nc.tensor.matmul(
                        zr_t[g][:, :PG], wslice(WZH), h16[g][:],
                        start=False, stop=True,
                    )

                a, b = go

                # -- ACT: sigmoid_r(a), then zbar(a) right away so group
                #    a's next-step zr openers (WAR on the zr PSUM tile)
                #    clear the PE FIFO long before h'(a) lands --
                r16, zb16 = [None] * G, [None] * G

                def sig_r(g):
                    rt = wk.tile([C, PG], f16, tag=f"r_{g}", name=f"r16_{g}")
                    nc.scalar.activation(
                        rt[:], zr_t[g][:, PG:], AF.Sigmoid, bias=bt[:, 1:2]
                    )
                    r16[g] = rt

                def sig_zbar(g):
                    zbt = wk.tile([C, PG], f16, tag=f"zb_{g}", name=f"zb16_{g}")
                    nc.scalar.activation(
                        zbt[:], zr_t[g][:, :PG], AF.Sigmoid,
                        bias=bt[:, 3:4], scale=-1.0,
                    )
                    zb16[g] = zbt

                sig_r(a)
                sig_zbar(a)
                sig_r(b)

                # group a's next-step z|r openers (zr_a fully consumed now)
                zr_next = [None] * G
                if x_next is not None:
                    zr_next_a = open_zr(x_next, [a])
                    zr_next[a] = zr_next_a[a]

                rh16 = [None] * G
                for g in go:
                    rh = wk.tile([C, PG], f16, tag=f"rh_{g}")
                    nc.vector.tensor_mul(rh[:], r16[g][:], h16[g][:])
                    rh16[g] = rh

                for g in go:
                    nc.tensor.matmul(
                        cp_t[g][:], wslice(WHH), rh16[g][:],
                        start=False, stop=True,
                    )

                # next step's c openers can run any time (double-buffered)
                cp_next = open_c(x_next, go) if x_next is not None else None

                # -- ACT: tanh(a), zbar(b), tanh(b) --
                c16 = [None] * G

                def tanh_c(g):
                    ct = wk.tile([C, PG], f16, tag=f"c16_{g}", name=f"c16_{g}")
                    nc.scalar.activation(
                        ct[:], cp_t[g][:], AF.Tanh, bias=bt[:, 2:3]
                    )
                    c16[g] = ct

                tanh_c(a)
                sig_zbar(b)
                tanh_c(b)

                # group b's next-step z|r openers
                if x_next is not None:
                    zr_next_b = open_zr(x_next, [b])
                    zr_next[b] = zr_next_b[b]

                # -- DVE tail, group-major so the first group's v/add
                #    never queue behind the second group's u/z (whose zbar
                #    lands later on the ACT stream) --
                for g in go:
                    ut = wk.tile([C, PG], f16, tag=f"u_{g}")
                    nc.vector.tensor_mul(ut[:], zb16[g][:], h16[g][:])
                    zt = wk.tile([C, PG], f16, tag=f"z_{g}")
                    nc.vector.tensor_scalar(
                        zt[:], zb16[g][:], -1.0, 1.0,
                        mybir.AluOpType.mult, mybir.AluOpType.add,
                    )
                    v16 = wk.tile([C, PG], f16, tag=f"v_{g}")
                    nc.vector.tensor_mul(v16[:], zt[:], c16[g][:])
                    n16 = spool.tile([C, PG], f16, tag=f"h16_{g}")
                    nc.vector.tensor_add(n16[:], ut[:], v16[:])
                    h16[g] = n16
                    nc.sync.dma_start(
                        o_ap[t, :, g * PG : (g + 1) * PG], n16[:]
                    )

                x_tiles.pop(t - 1, None)
                if x_next is not None:
                    zr_t, cp_t = zr_next, cp_next

    nc.compile()
    return nc


def _get_prog():
    global _PROG
    if _PROG is None:
        _PROG = _build()
    return _PROG


def _make_in_maps(video, Wz, bz, Wr, br, Wh, bh):
    w6 = np.concatenate(
        [
            Wz[:, :C].T, Wr[:, :C].T, Wh[:, :C].T,
            Wz[:, C:].T, Wr[:, C:].T, Wh[:, C:].T,
        ],
        axis=1,
    ).astype(np.float16)
    b3 = np.stack([bz, br, bh, -bz], axis=1).astype(np.float32)
    in_maps = []
    for core in range(NCORES):
        b_, q = divmod(core, 4)
        xs = np.ascontiguousarray(
            video[b_, :, :, q * HQ : (q + 1) * HQ, :]
        ).reshape(T, C, P).astype(np.float16)
        in_maps.append({"x_seq": xs, "wmats": w6, "biases": b3})
    return in_maps


def kernel(video, Wz, bz, Wr, br, Wh, bh):
    _ensure_paths()
    from concourse.bass_utils import run_bass_kernel_spmd

    video = np.asarray(video, dtype=np.float32)
    nc = _get_prog()
    in_maps = _make_in_maps(video, Wz, bz, Wr, br, Wh, bh)
    res = run_bass_kernel_spmd(nc, in_maps, list(range(NCORES)))

    out = np.empty((B, T, C, H, W), np.float32)
    for core in range(NCORES):
        b_, q = divmod(core, 4)
        out[b_, :, :, q * HQ : (q + 1) * HQ, :] = np.asarray(
            res.results[core]["out_seq"]
        ).astype(np.float32).reshape(T, C, HQ, W)
    return out


# revision 13
# speedup vs baseline: 1.0377x; 1.0377x over previous
"""ConvGRU Trainium2 kernel (v11).

video [B=2, T=16, C=128, H=64, W=64] f32; 1x1-conv GRU over T.
Sharding: data-parallel over (B x H/16) -> 8 cores, each core owns
P = 16*64 = 1024 pixels for all T; weights replicated.

Per core, per timestep (pixels on the free dim, channels on partitions):
    zr_pre = [Wzx@x + Wzh@h | Wrx@x + Wrh@h]      (PE, fp16 in / fp32 psum)
    z = sigmoid(zr_pre[:P] + bz); r = sigmoid(zr_pre[P:] + br)   (ACT)
    rh = r * h                                     (DVE)
    c = tanh(Whx@x + Whh@rh + bh)                  (PE + ACT)
    h' = u + v,  u = zbar*h,  v = z*c,  zbar = sigmoid(-pre_z)

G=2 pixel groups form two independent recurrence chains that
interleave on the engines.  The Scalar (ACT) engine is the pacing
resource: 6 sigmoid/tanh ops x ~690ns = 4.13us/step of streaming.
The remaining slack is the serial tail between the last tanh and the
next step's first r-sigmoid.  Structure choices that close it:

  - The next step's r-gate close is DISTRIBUTED over h' = u + v:
        pre_r(t+1) += Wrh@u(t)   (issues mid-step, u is ready early)
        pre_r(t+1) += Wrh@v(t)   (right after v -- the h' add leaves
                                  the sigmoid critical path entirely)
    The z-gate close stays a single Wzh@h' (zbar sits early in the
    next step's ACT stream, so it has slack).
  - Each group's zbar runs immediately after its own r-sigmoid, so
    that group's next-step zr openers (WAR on the single-buffered zr
    PSUM tile) clear the PE FIFO long before the r-closes arrive.
  - DVE tail is group-major (u,z,v,add per group): the first group's
    v/add never queue behind the second group's u/z, whose zbar lands
    later on the ACT stream.
  - t=0 shortcut: h0 == 0, so closers, r-sigmoid and rh are skipped.
  - fp16 everywhere: bf16 measures uniformly slower on this stack
    (ACTIVATE 687->823ns, TT 423->508ns); fp16 matmuls already
    pipeline at the 216ns/MM N=512 roofline.
  - Output DMAs + x prefetches share the sync HW queue (prefetch two
    steps ahead; DMA landing latency is about one step period).
    The gpsimd queue only carries the h-side weight load: anything
    more pays its ~3.6us dge_drain in the kernel tail.
  - PE warmup matmuls run against a memset tile (no weight-DMA
    dependency) flipping the HAM clock gate during the initial DMAs.

Numerics: fp16 matmul inputs/gates/state, fp32 PSUM accum + fp32 bias.
"""

import os
import sys

import numpy as np

B, T, C, H, W = 2, 16, 128, 64, 64
NCORES = 8
HQ = H // 4          # 16 rows of H per core (4 H-slices x 2 batches = 8 cores)
P = HQ * W           # 1024 pixels per core
G = 2                # pixel groups per step (independent recurrence chains)
PG = P // G          # 512 pixels per group

_PROG = None


def _ensure_paths():
    for p in ("/opt/trn_rl_repo",):
        if p not in sys.path and os.path.isdir(p):
            sys.path.append(p)


def _build():
    _ensure_paths()
    import concourse.bacc as bacc
    import concourse.tile as tile
    from concourse import mybir

    f32 = mybir.dt.float32
    f16 = mybir.dt.float16
    AF = mybir.ActivationFunctionType

    nc = bacc.Bacc(
        "TRN2", target_bir_lowering=False, debug=False, num_devices=NCORES
    )
    x_dram = nc.dram_tensor("x_seq", [T, C, P], f16, kind="ExternalInput")
    w_dram = nc.dram_tensor("wmats", [C, 6 * C], f16, kind="ExternalInput")
    b_dram = nc.dram_tensor("biases", [C, 4], f32, kind="ExternalInput")
    o_dram = nc.dram_tensor("out_seq", [T, C, P], f16, kind="ExternalOutput")

    x_ap = x_dram.ap()
    w_ap = w_dram.ap()
    b_ap = b_dram.ap()
    o_ap = o_dram.ap()

    # weight order in wmats: x-side first so its DMA can land first
    WZX, WRX, WHX, WZH, WRH, WHH = range(6)

    with tile.TileContext(nc) as tc:
        with (
            tc.tile_pool(name="consts", bufs=1) as consts,
            tc.tile_pool(name="xin", bufs=4) as xpool,
            tc.tile_pool(name="state", bufs=4) as spool,
            tc.tile_pool(name="work", bufs=3) as wk,
            tc.tile_pool(name="ps", bufs=1, space="PSUM") as ps,
        ):
            wt = consts.tile([C, 6 * C], f16)
            bt = consts.tile([C, 4], f32)
            nc.sync.dma_start(bt[:], b_ap[:])
            nc.gpsimd.dma_start(wt[:, 3 * C :], w_ap[:, 3 * C :])

            def wslice(i):
                return wt[:, i * C : (i + 1) * C]

            def load_x(t):
                xt = xpool.tile([C, P], f16, tag="x")
                nc.sync.dma_start(xt[:], x_ap[t])
                return xt

            x_tiles = {0: load_x(0)}
            nc.sync.dma_start(wt[:, : 3 * C], w_ap[:, : 3 * C])
            x_tiles[1] = load_x(1)
            x_tiles[2] = load_x(2)

            # -- warmup: ramp the PE clock gate with matmuls that only
            #    depend on a memset tile, while the input DMAs fly --
            warm16 = wk.tile([C, PG], f16, tag="warm")
            nc.vector.memset(warm16[:], 0.0)
            cwarm = [None, None]
            for g in range(G):
                cwarm[g] = ps.tile(
                    [C, PG], f32, tag=f"c_{g}", bufs=2, name=f"cwarm_{g}"
                )
            for i in range(6):
                nc.tensor.matmul(
                    cwarm[i % 2][:], warm16[:, :C], warm16[:],
                    start=True, stop=True,
                )
            # preload the ACT sigmoid/tanh table early
            wtmp = wk.tile([C, PG], f16, tag="scratch")
            nc.scalar.activation(
                wtmp[:], cwarm[0][:], AF.Sigmoid, bias=bt[:, 0:1]
            )

            def open_zr(xt, g):
                """Open one group's z|r accumulation with the x-side."""
                xs = xt[:, g * PG : (g + 1) * PG]
                zrt = ps.tile([C, 2 * PG], f32, tag=f"zr_{g}", bufs=1,
                              name=f"zr_t{g}")
                nc.tensor.matmul(
                    zrt[:, PG:], wslice(WRX), xs, start=True, stop=False
                )
                nc.tensor.matmul(
                    zrt[:, :PG], wslice(WZX), xs, start=True, stop=False
                )
                return zrt

            def open_c(xt, g):
                xs = xt[:, g * PG : (g + 1) * PG]
                cp = ps.tile([C, PG], f32, tag=f"c_{g}", bufs=2,
                             name=f"c_t{g}")
                nc.tensor.matmul(
                    cp[:], wslice(WHX), xs, start=True, stop=False
                )
                return cp

            # ---- t = 0: h0 == 0, so no closers / r-gate / rh ----
            x0 = x_tiles[0]
            zr0 = [None, None]
            for g in range(G):
                zrt = ps.tile([C, 2 * PG], f32, tag=f"zr_{g}", name=f"zr0_{g}")
                nc.tensor.matmul(
                    zrt[:, :PG], wslice(WZX), x0[:, g * PG : (g + 1) * PG],
                    start=True, stop=True,
                )
                zr0[g] = zrt
            c0 = [None, None]
            for g in range(G):
                cp = ps.tile([C, PG], f32, tag=f"c_{g}", bufs=2)
                nc.tensor.matmul(
                    cp[:], wslice(WHX), x0[:, g * PG : (g + 1) * PG],
                    start=True, stop=True,
                )
                c0[g] = cp
            h16 = [None, None]
            for g in range(G):
                zt = wk.tile([C, PG], f16, tag=f"zb_{g}")
                nc.scalar.activation(
                    zt[:], zr0[g][:, :PG], AF.Sigmoid, bias=bt[:, 0:1]
                )
                ct = wk.tile([C, PG], f16, tag=f"c16_{g}")
                nc.scalar.activation(ct[:], c0[g][:], AF.Tanh, bias=bt[:, 2:3])
                ht = spool.tile([C, PG], f16, tag=f"h16_{g}")
                nc.vector.tensor_mul(ht[:], zt[:], ct[:])
                h16[g] = ht
                nc.sync.dma_start(
                    o_ap[0, :, g * PG : (g + 1) * PG], ht[:]
                )

            # open + close t=1's zr with h'(0) (plain single closes)
            x1 = x_tiles[1]
            zr_t = [None, None]
            cp_t = [None, None]
            for g in range(G):
                zr_t[g] = open_zr(x1, g)
                nc.tensor.matmul(
                    zr_t[g][:, PG:], wslice(WRH), h16[g][:],
                    start=False, stop=True,
                )
                nc.tensor.matmul(
                    zr_t[g][:, :PG], wslice(WZH), h16[g][:],
                    start=False, stop=True,
                )
                cp_t[g] = open_c(x1, g)

            # ---- steady steps t = 1..T-1 ----
            # zr_t arrives FULLY CLOSED (r closed via Wrh@u + Wrh@v of the
            # previous step's blend; z closed via Wzh@h')
            for t in range(1, T):
                go = [0, 1] if t % 2 == 1 else [1, 0]
                a, b = go
                x_next = x_tiles.get(t + 1)
                if t + 2 < T:
                    x_tiles[t + 2] = load_x(t + 2)

                r16, zb16 = [None] * G, [None] * G

                def sig_r(g):
                    rt = wk.tile([C, PG], f16, tag=f"r_{g}", name=f"r16_{g}")
                    nc.scalar.activation(
                        rt[:], zr_t[g][:, PG:], AF.Sigmoid, bias=bt[:, 1:2]
                    )
                    r16[g] = rt

                def sig_zbar(g):
                    zbt = wk.tile([C, PG], f16, tag=f"zb_{g}", name=f"zb16_{g}")
                    nc.scalar.activation(
                        zbt[:], zr_t[g][:, :PG], AF.Sigmoid,
                        bias=bt[:, 3:4], scale=-1.0,
                    )
                    zb16[g] = zbt

                sig_r(a)
                sig_zbar(a)
                sig_r(b)

                # group a's next-step z|r openers (zr_a fully consumed)
                zr_next = [None] * G
                if x_next is not None:
                    zr_next[a] = open_zr(x_next, a)

                rh16 = [None] * G
                for g in go:
                    rh = wk.tile([C, PG], f16, tag=f"rh_{g}")
                    nc.vector.tensor_mul(rh[:], r16[g][:], h16[g][:])
                    rh16[g] = rh

                for g in go:
                    nc.tensor.matmul(
                        cp_t[g][:], wslice(WHH), rh16[g][:],
                        start=False, stop=True,
                    )

                cp_next = [None] * G
                if x_next is not None:
                    for g in go:
                        cp_next[g] = open_c(x_next, g)

                c16 = [None] * G

                def tanh_c(g):
                    ct = wk.tile([C, PG], f16, tag=f"c16_{g}", name=f"c16_{g}")
                    nc.scalar.activation(
                        ct[:], cp_t[g][:], AF.Tanh, bias=bt[:, 2:3]
                    )
                    c16[g] = ct

                def blend(g):
                    """u,z early; v + h'-add after tanh; the next step's
                    r-close rides u and v so the sigmoid path never waits
                    for the add."""
                    ut = wk.tile([C, PG], f16, tag=f"u_{g}", name=f"u16_{g}")
                    nc.vector.tensor_mul(ut[:], zb16[g][:], h16[g][:])
                    zt = wk.tile([C, PG], f16, tag=f"z_{g}", name=f"z16_{g}")
                    nc.vector.tensor_scalar(
                        zt[:], zb16[g][:], -1.0, 1.0,
                        mybir.AluOpType.mult, mybir.AluOpType.add,
                    )
                    if zr_next[g] is not None:
                        nc.tensor.matmul(
                            zr_next[g][:, PG:], wslice(WRH), ut[:],
                            start=False, stop=False,
                        )
                    v16 = wk.tile([C, PG], f16, tag=f"v_{g}", name=f"v16_{g}")
                    nc.vector.tensor_mul(v16[:], zt[:], c16[g][:])
                    if zr_next[g] is not None:
                        nc.tensor.matmul(
                            zr_next[g][:, PG:], wslice(WRH), v16[:],
                            start=False, stop=True,
                        )
                    n16 = spool.tile([C, PG], f16, tag=f"h16_{g}",
                                     name=f"h16n_{g}")
                    nc.vector.tensor_add(n16[:], ut[:], v16[:])
                    h16[g] = n16
                    if zr_next[g] is not None:
                        nc.tensor.matmul(
                            zr_next[g][:, :PG], wslice(WZH), n16[:],
                            start=False, stop=True,
                        )
                    nc.sync.dma_start(
                        o_ap[t, :, g * PG : (g + 1) * PG], n16[:]
                    )

                tanh_c(a)
                sig_zbar(b)
                blend(a)
                tanh_c(b)
                if x_next is not None:
                    zr_next[b] = open_zr(x_next, b)
                blend(b)

                x_tiles.pop(t - 1, None)
                if x_next is not None:
                    zr_t, cp_t = zr_next, cp_next

    nc.compile()
    return nc


def _get_prog():
    global _PROG
    if _PROG is None:
        _PROG = _build()
    return _PROG


def _make_in_maps(video, Wz, bz, Wr, br, Wh, bh):
    w6 = np.concatenate(
        [
            Wz[:, :C].T, Wr[:, :C].T, Wh[:, :C].T,
            Wz[:, C:].T, Wr[:, C:].T, Wh[:, C:].T,
        ],
        axis=1,
    ).astype(np.float16)
    b3 = np.stack([bz, br, bh, -bz], axis=1).astype(np.float32)
    in_maps = []
    for core in range(NCORES):
        b_, q = divmod(core, 4)
        xs = np.ascontiguousarray(
            video[b_, :, :, q * HQ : (q + 1) * HQ, :]
        ).reshape(T, C, P).astype(np.float16)
        in_maps.append({"x_seq": xs, "wmats": w6, "biases": b3})
    return in_maps


def kernel(video, Wz, bz, Wr, br, Wh, bh):
    _ensure_paths()
    from concourse.bass_utils import run_bass_kernel_spmd

    video = np.asarray(video, dtype=np.float32)
    nc = _get_prog()
    in_maps = _make_in_maps(video, Wz, bz, Wr, br, Wh, bh)
    res = run_bass_kernel_spmd(nc, in_maps, list(range(NCORES)))

    out = np.empty((B, T, C, H, W), np.float32)
    for core in range(NCORES):
        b_, q = divmod(core, 4)
        out[b_, :, :, q * HQ : (q + 1) * HQ, :] = np.asarray(
            res.results[core]["out_seq"]
        ).astype(np.float32).reshape(T, C, HQ, W)
    return out
